# revision 39
# baseline (speedup 1.0000x reference)
"""Modulated 1x1 conv (ModConv) on 8 Trainium2 NeuronCores.

out[b,o,h,w] = sum_c (style[b,c] * weight[o,c]) * x[b,c,h,w]

Strategy: pure data parallel over the batch — 2 samples per core. The
kernel is DMA-bound, so two levers dominate:

1. Bytes on the wire. x is cast to fp16 on the host (the harness gate
   is rel_err < 2e-2; fp16 end-to-end lands ~4e-4) and the output
   leaves the device as fp16 — ~10.5 MB/core instead of ~21 MB fp32.
   The PE stays full-rate (1 cycle/row) for fp16, same as fp32r.
2. DMA queue parallelism. Per-queue throughput caps well below the
   aggregate (measured: 2 queues ~390 GB/s, 3 queues ~540 GB/s), so
   all three DMA-capable rings (SP + ACT HWDGE, Pool SWDGE) carry x
   AND output chunks round-robin, ~3.5 MB/ring. Output DMAs are
   deferred by one sample so their compute-completion waits are
   already satisfied at issue — without this, a compute-gated out DMA
   head-of-line-blocks later x chunks queued behind it on the same
   engine sequencer (measured up to 2x slowdown). A 4th-6th queue via
   identity dma_gather on SWDGE queues 1-3 (gq>0 below) is
   CoreSim-correct but uncompilable here: the container's public-SDK
   walrus rejects DMAGatherAnt ("ISA wrong length"), so gq stays 0.

x is pre-transposed on the host to [qn, 128, KT, qw] per sample so
each chunk DMA reads one contiguous block (4 KB per-partition
descriptor lines). Per sample the kernel modulates the
(pre-transposed) weight with the style vector on DVE in fp32, rounds
to fp16, then runs the K=512 contraction as 4 PSUM-accumulated fp16
matmuls per 512-wide output tile; DVE downcasts PSUM->SBUF to fp16.

Per-core floor: PE 13.7 us busy, DMA ~10.5 MB across 3 rings. The
axon-tunnel slope bench draws 19.6-33 us for this config run-to-run
(device clock/p-state state between RPC calls); fp32 baseline was
60-62 us.
"""

import numpy as np

import concourse.bass as bass
import concourse.mybir as mybir
from concourse import library_config
from concourse.bass_utils import run_bass_kernel_spmd
from concourse.tile import TileContext

B, CIN, COUT, H, W = 16, 512, 128, 64, 64
HW = H * W
N_CORES = 8
BPC = B // N_CORES  # samples per core
P = 128
KT = CIN // P  # k-tiles per contraction
FP32 = mybir.dt.float32
FP16 = mybir.dt.float16

# This container's walrus (public-SDK build) accepts at most one sync
# wait command per instruction; Tile's sem assignment attaches one wait
# per depended-on proc. Hoist the excess onto dedicated wait
# instructions (the same InstEventSemaphore a bass `wait_ge` emits)
# immediately before the over-subscribed instruction on its own engine.
MAX_WAITS_PER_INST = 1


def _split_sync_waits(nc: bass.Bass, limit: int = MAX_WAITS_PER_INST) -> int:
    n_split = 0
    for f in nc.m.functions:
        for bb in f.blocks:
            out = []
            for ins in bb.instructions:
                si = getattr(ins, "sync_info", None)
                if si is not None and si.on_wait and len(si.on_wait) > limit:
                    waits = list(si.on_wait)
                    for w in waits[:-limit]:
                        n_split += 1
                        es = mybir.InstEventSemaphore(
                            name=f"{ins.name}-ws{n_split}",
                            opcode="EventSemaphore",
                            engine=ins.engine,
                            sync_info=mybir.SyncInfo(on_wait=[w], on_update=[]),
                        )
                        nc.register_instruction(es, overwrite=True)
                        out.append(es)
                    si.on_wait = waits[-limit:]
                out.append(ins)
            bb.instructions[:] = out
    return n_split


def build_kernel(
    reps: int = 1,
    bench_mode: bool = False,
    qn: int = 8,  # x DMA chunks per sample, each [128, KT, HW/qn] fp16
    x_bufs: int | None = None,
    psum_bufs: int = 4,
    o_bufs: int = 4,
    out_chunks: int = 2,  # output DMAs per sample
    x_engines: str = "sag",  # s=SP, a=ACT (HWDGE), g=Pool (SWDGE)
    out_engines: str = "s",  # round-robin over these for output DMAs
    defer_out: int = 1,  # defer out DMAs this many samples; join x round-robin
    xlay: str = "c",  # x DRAM layout: "c"=pre-transposed contiguous chunks,
    #                   "r"=[CIN, HW] with a strided rearrange on the DMA
    gq: int = 0,  # extra SWDGE queues (1..3) fed via identity dma_gather
) -> bass.Bass:
    """reps>1 replicates the whole per-sample pipeline in-program (same
    inputs, outputs rewritten) — used only by the bench to measure
    steady-state per-iteration time with per-call overhead cancelled.
    bench_mode writes the big output to internal DRAM and exposes only a
    4-byte token output, so per-call tunnel traffic is negligible."""
    qw = HW // qn
    ntile = min(512, qw)  # PSUM tile width (512 fp32 = one 2 KB bank)
    nt = HW // ntile
    if x_bufs is None:
        # One slot of slack beyond 2 samples in flight, so the HWDGE
        # rings never stall on a slot release (measured on the fp32
        # variant: +1 slot ~20 us faster; one more regresses again).
        x_bufs = 2 * qn + 1
    E = KT * qw  # elements per partition line of one chunk
    nc = bass.Bass(num_swdge_queues=1 + gq)
    # xlay="c": x arrives pre-transposed on the host to [BPC, qn, P, KT*qw]:
    # each chunk DMA reads one fully contiguous block with 4 KB+ per-partition
    # descriptor lines (vs 1 KB strided lines from a [CIN, HW] layout).
    if xlay == "c":
        x = nc.dram_tensor("x", [BPC, qn, P, E], FP16, kind="ExternalInput")
    else:
        x = nc.dram_tensor("x", [BPC, CIN, HW], FP16, kind="ExternalInput")
    # Identity gather indices (idx i read from [i % 16, i // 16]), host-filled.
    gidx = (
        nc.dram_tensor("gidx", [P, P // 16], mybir.dt.int16, kind="ExternalInput")
        if gq
        else None
    )
    styleT = nc.dram_tensor("styleT", [CIN, BPC], FP32, kind="ExternalInput")
    wT = nc.dram_tensor("wT", [CIN, COUT], FP32, kind="ExternalInput")
    if bench_mode:
        out = nc.dram_tensor("out_scratch", [BPC, COUT, HW], FP16)
        token = nc.dram_tensor("token", [1, 1], FP16, kind="ExternalOutput")
    else:
        out = nc.dram_tensor("out", [BPC, COUT, HW], FP16, kind="ExternalOutput")
        token = None

    # x rings: plain dma_start queues (SP/ACT HWDGE, Pool SWDGE q0) plus
    # optional extra Pool SWDGE queues 1..gq driven by identity dma_gather.
    eng_map = {"s": nc.sync, "a": nc.scalar, "g": nc.gpsimd}
    x_rings = [("p", eng_map[c]) for c in x_engines] + [
        ("q", k + 1) for k in range(gq)
    ]
    out_dma_engines = [eng_map[c] for c in out_engines]

    with TileContext(nc) as tc:
        with (
            tc.tile_pool(name="consts", bufs=1) as cpool,
            tc.tile_pool(name="xs", bufs=x_bufs) as xpool,
            tc.tile_pool(name="os", bufs=o_bufs) as opool,
            tc.tile_pool(name="ps", bufs=psum_bufs, space="PSUM") as pspool,
        ):
            wT_sb = cpool.tile([P, KT, COUT], FP32)
            nc.sync.dma_start(out=wT_sb[:], in_=wT[:].rearrange("(t p) o -> p t o", p=P))
            sT_sb = cpool.tile([P, KT, BPC], FP32)
            nc.scalar.dma_start(
                out=sT_sb[:], in_=styleT[:].rearrange("(t p) b -> p t b", p=P)
            )
            # Per-sample modulated (transposed) weights: mw[p, b, t, o],
            # computed in fp32 on DVE, rounded to fp16 on the write.
            mw_sb = cpool.tile([P, BPC, KT, COUT], FP16)
            for b in range(BPC):
                for t in range(KT):
                    nc.vector.tensor_scalar_mul(
                        mw_sb[:, b, t, :], wT_sb[:, t, :], sT_sb[:, t, b : b + 1]
                    )
            if gq:
                gidx_sb = cpool.tile([P, P // 16], mybir.dt.int16)
                nc.sync.dma_start(out=gidx_sb[:], in_=gidx[:])
                # DMAGatherAnt lives in the mlp/attnmlp Q7 libraries.
                nc.gpsimd.load_library(library_config.mlp)

            oev = nt // out_chunks  # n-tiles per output DMA
            dma_i = 0
            out_i = 0
            pending = []  # deferred out DMAs: (b, lo, hi, ot)
            for _rep in range(reps):
                for b in range(BPC):
                    # Drain deferred out DMAs (their producing copies are
                    # >=defer_out samples old, so the sem wait is already
                    # satisfied — no head-of-line blocking on the ring),
                    # interleaved round-robin with this sample's x chunks.
                    issue_now = []
                    if defer_out and len(pending) > defer_out * out_chunks:
                        issue_now = pending[: len(pending) - defer_out * out_chunks]
                        pending = pending[len(pending) - defer_out * out_chunks :]
                    # One DMA per HW-chunk carrying all 4 k-tiles.
                    xq = []
                    for q in range(qn):
                        if issue_now and q % 2 == 0:
                            ob, lo, hi, oot = issue_now.pop(0)
                            oeng = out_dma_engines[out_i % len(out_dma_engines)]
                            out_i += 1
                            oeng.dma_start(out=out[ob, :, lo:hi], in_=oot[:, lo:hi])
                        xt = xpool.tile([P, 1, E], FP16, tag="xt")
                        kind, v = x_rings[dma_i % len(x_rings)]
                        dma_i += 1
                        if xlay != "c":
                            v.dma_start(
                                out=xt[:, 0, :].rearrange("p (t n) -> p t n", t=KT),
                                in_=x[b, :, q * qw : (q + 1) * qw].rearrange(
                                    "(t p) n -> p t n", p=P
                                ),
                            )
                        elif kind == "p":
                            v.dma_start(out=xt[:, 0, :], in_=x[b, q])
                        else:
                            nc.gpsimd.dma_gather(
                                xt[:], x[b, q], gidx_sb[:], P, P, E,
                                elem_step=E, queue_num=v,
                            )
                        xq.append(xt)
                    for ob, lo, hi, oot in issue_now:
                        oeng = out_dma_engines[out_i % len(out_dma_engines)]
                        out_i += 1
                        oeng.dma_start(out=out[ob, :, lo:hi], in_=oot[:, lo:hi])

                    ot = opool.tile([P, HW], FP16, tag="ot")
                    for n in range(nt):
                        ps = pspool.tile([P, ntile], FP32, tag="ps")
                        q, j = divmod(n, max(nt // qn, 1))
                        for t in range(KT):
                            lo_r = t * qw + j * ntile
                            nc.tensor.matmul(
                                ps[:],
                                mw_sb[:, b, t, :],
                                xq[q][:, 0, lo_r : lo_r + ntile],
                                start=(t == 0),
                                stop=(t == KT - 1),
                            )
                        nc.vector.tensor_copy(
                            out=ot[:, n * ntile : (n + 1) * ntile], in_=ps[:]
                        )
                        if (n + 1) % oev == 0:
                            lo = (n + 1 - oev) * ntile
                            hi = (n + 1) * ntile
                            if defer_out:
                                pending.append((b, lo, hi, ot))
                            else:
                                oeng = out_dma_engines[out_i % len(out_dma_engines)]
                                out_i += 1
                                oeng.dma_start(out=out[b, :, lo:hi], in_=ot[:, lo:hi])
            for ob, lo, hi, oot in pending:
                oeng = out_dma_engines[out_i % len(out_dma_engines)]
                out_i += 1
                oeng.dma_start(out=out[ob, :, lo:hi], in_=oot[:, lo:hi])
            if token is not None:
                # On sync, not Pool: Pool's SWDGE sem lanes are queue-locked
                # and must keep their periodic gather pattern when gq > 0.
                nc.sync.dma_start(out=token[:], in_=mw_sb[:1, 0, 0, :1])

    _split_sync_waits(nc)
    return nc


_NC_CACHE: bass.Bass | None = None


def _get_nc() -> bass.Bass:
    global _NC_CACHE
    if _NC_CACHE is None:
        _NC_CACHE = build_kernel()
    return _NC_CACHE


def make_in_maps(
    x: np.ndarray, style: np.ndarray, weight: np.ndarray, qn: int = 8, xlay: str = "c"
):
    qw = HW // qn
    # xlay="c": [B, CIN, HW] -> fp16 [B, qn, P, KT*qw]: chunk q / partition
    # p holds x[b, t*P + p, q*qw : (q+1)*qw] at offset t*qw — the layout
    # each chunk DMA consumes as one contiguous block.
    if xlay == "c":
        x_t = (
            np.asarray(x, dtype=np.float32)
            .reshape(B, KT, P, qn, qw)
            .transpose(0, 3, 2, 1, 4)
            .reshape(B, qn, P, KT * qw)
            .astype(np.float16)
        )
    else:
        x_t = np.asarray(x, dtype=np.float32).reshape(B, CIN, HW).astype(np.float16)
    # Identity gather indices: idx i is read from [i % 16, i // 16].
    gidx = np.zeros((P, P // 16), dtype=np.int16)
    for j in range(P // 16):
        gidx[:16, j] = np.arange(16, dtype=np.int16) + 16 * j
    styleT = np.ascontiguousarray(np.asarray(style, dtype=np.float32).T)  # [CIN, B]
    wT = np.ascontiguousarray(np.asarray(weight, dtype=np.float32).T)  # [CIN, COUT]
    in_maps = []
    for c in range(N_CORES):
        sl = slice(c * BPC, (c + 1) * BPC)
        in_maps.append(
            {
                "x": np.ascontiguousarray(x_t[sl]),
                "styleT": np.ascontiguousarray(styleT[:, sl]),
                "wT": wT,
                "gidx": gidx,
            }
        )
    return in_maps


def gather_out(results) -> np.ndarray:
    out = np.empty((B, COUT, H, W), dtype=np.float32)
    for c in range(N_CORES):
        out[c * BPC : (c + 1) * BPC] = (
            results[c]["out"].astype(np.float32).reshape(BPC, COUT, H, W)
        )
    return out


def kernel(x: np.ndarray, style: np.ndarray, weight: np.ndarray) -> np.ndarray:
    nc = _get_nc()
    in_maps = make_in_maps(x, style, weight)
    res = run_bass_kernel_spmd(nc, in_maps, core_ids=list(range(N_CORES)))
    return gather_out(res.results)


# revision 40
# speedup vs baseline: 1.7212x; 1.7212x over previous
"""Modulated 1x1 conv (ModConv) on 8 Trainium2 NeuronCores.

out[b,o,h,w] = sum_c (style[b,c] * weight[o,c]) * x[b,c,h,w]

Strategy: pure data parallel over the batch — 2 samples per core. The
kernel is DMA-bound, so two levers dominate:

1. Bytes on the wire. x is cast to fp16 on the host (the harness gate
   is rel_err < 2e-2; fp16 end-to-end lands ~4e-4) and the output
   leaves the device as fp16 — ~10.5 MB/core instead of ~21 MB fp32.
   The PE stays full-rate (1 cycle/row) for fp16, same as fp32r.
2. DMA queue parallelism. Per-queue throughput caps well below the
   aggregate (measured: 2 queues ~390 GB/s, 3 queues ~540 GB/s), so
   all three DMA-capable rings (SP + ACT HWDGE, Pool SWDGE) carry x
   AND output chunks round-robin, ~3.5 MB/ring. Output DMAs are
   deferred by one sample so their compute-completion waits are
   already satisfied at issue — without this, a compute-gated out DMA
   head-of-line-blocks later x chunks queued behind it on the same
   engine sequencer (measured up to 2x slowdown). A 4th-6th queue via
   identity dma_gather on SWDGE queues 1-3 (gq>0 below) is
   CoreSim-correct but uncompilable here: the container's public-SDK
   walrus rejects DMAGatherAnt ("ISA wrong length"), so gq stays 0.

x is pre-transposed on the host to [qn, 128, KT, qw] per sample so
each chunk DMA reads one contiguous block (4 KB per-partition
descriptor lines). Per sample the kernel modulates the
(pre-transposed) weight with the style vector on DVE in fp32, rounds
to fp16, then runs the K=512 contraction as 4 PSUM-accumulated fp16
matmuls per 512-wide output tile; DVE downcasts PSUM->SBUF to fp16.

Per-core floor: PE 13.7 us busy, DMA ~10.5 MB across 3 rings. The
axon-tunnel slope bench draws 19.6-33 us for this config run-to-run
(device clock/p-state state between RPC calls); fp32 baseline was
60-62 us.
"""

import numpy as np

import concourse.bass as bass
import concourse.mybir as mybir
from concourse import library_config
from concourse.bass_utils import run_bass_kernel_spmd
from concourse.tile import TileContext

B, CIN, COUT, H, W = 16, 512, 128, 64, 64
HW = H * W
N_CORES = 8
BPC = B // N_CORES  # samples per core
P = 128
KT = CIN // P  # k-tiles per contraction
FP32 = mybir.dt.float32
FP16 = mybir.dt.float16

# This container's walrus (public-SDK build) accepts at most one sync
# wait command per instruction; Tile's sem assignment attaches one wait
# per depended-on proc. Hoist the excess onto dedicated wait
# instructions (the same InstEventSemaphore a bass `wait_ge` emits)
# immediately before the over-subscribed instruction on its own engine.
MAX_WAITS_PER_INST = 1


def _split_sync_waits(nc: bass.Bass, limit: int = MAX_WAITS_PER_INST) -> int:
    n_split = 0
    for f in nc.m.functions:
        for bb in f.blocks:
            out = []
            for ins in bb.instructions:
                si = getattr(ins, "sync_info", None)
                if si is not None and si.on_wait and len(si.on_wait) > limit:
                    waits = list(si.on_wait)
                    for w in waits[:-limit]:
                        n_split += 1
                        es = mybir.InstEventSemaphore(
                            name=f"{ins.name}-ws{n_split}",
                            opcode="EventSemaphore",
                            engine=ins.engine,
                            sync_info=mybir.SyncInfo(on_wait=[w], on_update=[]),
                        )
                        nc.register_instruction(es, overwrite=True)
                        out.append(es)
                    si.on_wait = waits[-limit:]
                out.append(ins)
            bb.instructions[:] = out
    return n_split


def build_kernel(
    reps: int = 1,
    bench_mode: bool = False,
    qn: int = 8,  # x DMA chunks per sample, each [128, KT, HW/qn] fp16
    x_bufs: int | None = None,
    psum_bufs: int = 4,
    o_bufs: int = 4,
    out_chunks: int = 2,  # output DMAs per sample
    x_engines: str = "sag",  # s=SP, a=ACT (HWDGE), g=Pool (SWDGE)
    out_engines: str = "sag",  # round-robin over these for output DMAs
    defer_out: int = 1,  # defer out DMAs this many samples; join x round-robin
    xlay: str = "c",  # x DRAM layout: "c"=pre-transposed contiguous chunks,
    #                   "r"=[CIN, HW] with a strided rearrange on the DMA
    gq: int = 0,  # extra SWDGE queues (1..3) fed via identity dma_gather
) -> bass.Bass:
    """reps>1 replicates the whole per-sample pipeline in-program (same
    inputs, outputs rewritten) — used only by the bench to measure
    steady-state per-iteration time with per-call overhead cancelled.
    bench_mode writes the big output to internal DRAM and exposes only a
    4-byte token output, so per-call tunnel traffic is negligible."""
    qw = HW // qn
    ntile = min(512, qw)  # PSUM tile width (512 fp32 = one 2 KB bank)
    nt = HW // ntile
    if x_bufs is None:
        # One slot of slack beyond 2 samples in flight, so the HWDGE
        # rings never stall on a slot release (measured on the fp32
        # variant: +1 slot ~20 us faster; one more regresses again).
        x_bufs = 2 * qn + 1
    E = KT * qw  # elements per partition line of one chunk
    nc = bass.Bass(num_swdge_queues=1 + gq)
    # xlay="c": x arrives pre-transposed on the host to [BPC, qn, P, KT*qw]:
    # each chunk DMA reads one fully contiguous block with 4 KB+ per-partition
    # descriptor lines (vs 1 KB strided lines from a [CIN, HW] layout).
    if xlay == "c":
        x = nc.dram_tensor("x", [BPC, qn, P, E], FP16, kind="ExternalInput")
    else:
        x = nc.dram_tensor("x", [BPC, CIN, HW], FP16, kind="ExternalInput")
    # Identity gather indices (idx i read from [i % 16, i // 16]), host-filled.
    gidx = (
        nc.dram_tensor("gidx", [P, P // 16], mybir.dt.int16, kind="ExternalInput")
        if gq
        else None
    )
    styleT = nc.dram_tensor("styleT", [CIN, BPC], FP32, kind="ExternalInput")
    wT = nc.dram_tensor("wT", [CIN, COUT], FP32, kind="ExternalInput")
    if bench_mode:
        out = nc.dram_tensor("out_scratch", [BPC, COUT, HW], FP16)
        token = nc.dram_tensor("token", [1, 1], FP16, kind="ExternalOutput")
    else:
        out = nc.dram_tensor("out", [BPC, COUT, HW], FP16, kind="ExternalOutput")
        token = None

    # x rings: plain dma_start queues (SP/ACT HWDGE, Pool SWDGE q0) plus
    # optional extra Pool SWDGE queues 1..gq driven by identity dma_gather.
    eng_map = {"s": nc.sync, "a": nc.scalar, "g": nc.gpsimd}
    x_rings = [("p", eng_map[c]) for c in x_engines] + [
        ("q", k + 1) for k in range(gq)
    ]
    out_dma_engines = [eng_map[c] for c in out_engines]

    with TileContext(nc) as tc:
        with (
            tc.tile_pool(name="consts", bufs=1) as cpool,
            tc.tile_pool(name="xs", bufs=x_bufs) as xpool,
            tc.tile_pool(name="os", bufs=o_bufs) as opool,
            tc.tile_pool(name="ps", bufs=psum_bufs, space="PSUM") as pspool,
        ):
            wT_sb = cpool.tile([P, KT, COUT], FP32)
            nc.sync.dma_start(out=wT_sb[:], in_=wT[:].rearrange("(t p) o -> p t o", p=P))
            sT_sb = cpool.tile([P, KT, BPC], FP32)
            nc.scalar.dma_start(
                out=sT_sb[:], in_=styleT[:].rearrange("(t p) b -> p t b", p=P)
            )
            # Per-sample modulated (transposed) weights: mw[p, b, t, o],
            # computed in fp32 on DVE, rounded to fp16 on the write.
            mw_sb = cpool.tile([P, BPC, KT, COUT], FP16)
            for b in range(BPC):
                for t in range(KT):
                    nc.vector.tensor_scalar_mul(
                        mw_sb[:, b, t, :], wT_sb[:, t, :], sT_sb[:, t, b : b + 1]
                    )
            if gq:
                gidx_sb = cpool.tile([P, P // 16], mybir.dt.int16)
                nc.sync.dma_start(out=gidx_sb[:], in_=gidx[:])
                # DMAGatherAnt lives in the mlp/attnmlp Q7 libraries.
                nc.gpsimd.load_library(library_config.mlp)

            oev = nt // out_chunks  # n-tiles per output DMA
            dma_i = 0
            out_i = 0
            pending = []  # deferred out DMAs: (b, lo, hi, ot)
            for _rep in range(reps):
                for b in range(BPC):
                    # Drain deferred out DMAs (their producing copies are
                    # >=defer_out samples old, so the sem wait is already
                    # satisfied — no head-of-line blocking on the ring),
                    # interleaved round-robin with this sample's x chunks.
                    issue_now = []
                    if defer_out and len(pending) > defer_out * out_chunks:
                        issue_now = pending[: len(pending) - defer_out * out_chunks]
                        pending = pending[len(pending) - defer_out * out_chunks :]
                    # One DMA per HW-chunk carrying all 4 k-tiles.
                    xq = []
                    for q in range(qn):
                        if issue_now and q % 2 == 0:
                            ob, lo, hi, oot = issue_now.pop(0)
                            oeng = out_dma_engines[out_i % len(out_dma_engines)]
                            out_i += 1
                            oeng.dma_start(out=out[ob, :, lo:hi], in_=oot[:, lo:hi])
                        xt = xpool.tile([P, 1, E], FP16, tag="xt")
                        kind, v = x_rings[dma_i % len(x_rings)]
                        dma_i += 1
                        if xlay != "c":
                            v.dma_start(
                                out=xt[:, 0, :].rearrange("p (t n) -> p t n", t=KT),
                                in_=x[b, :, q * qw : (q + 1) * qw].rearrange(
                                    "(t p) n -> p t n", p=P
                                ),
                            )
                        elif kind == "p":
                            v.dma_start(out=xt[:, 0, :], in_=x[b, q])
                        else:
                            nc.gpsimd.dma_gather(
                                xt[:], x[b, q], gidx_sb[:], P, P, E,
                                elem_step=E, queue_num=v,
                            )
                        xq.append(xt)
                    for ob, lo, hi, oot in issue_now:
                        oeng = out_dma_engines[out_i % len(out_dma_engines)]
                        out_i += 1
                        oeng.dma_start(out=out[ob, :, lo:hi], in_=oot[:, lo:hi])

                    ot = opool.tile([P, HW], FP16, tag="ot")
                    for n in range(nt):
                        ps = pspool.tile([P, ntile], FP32, tag="ps")
                        q, j = divmod(n, max(nt // qn, 1))
                        for t in range(KT):
                            lo_r = t * qw + j * ntile
                            nc.tensor.matmul(
                                ps[:],
                                mw_sb[:, b, t, :],
                                xq[q][:, 0, lo_r : lo_r + ntile],
                                start=(t == 0),
                                stop=(t == KT - 1),
                            )
                        nc.vector.tensor_copy(
                            out=ot[:, n * ntile : (n + 1) * ntile], in_=ps[:]
                        )
                        if (n + 1) % oev == 0:
                            lo = (n + 1 - oev) * ntile
                            hi = (n + 1) * ntile
                            if defer_out:
                                pending.append((b, lo, hi, ot))
                            else:
                                oeng = out_dma_engines[out_i % len(out_dma_engines)]
                                out_i += 1
                                oeng.dma_start(out=out[b, :, lo:hi], in_=ot[:, lo:hi])
            for ob, lo, hi, oot in pending:
                oeng = out_dma_engines[out_i % len(out_dma_engines)]
                out_i += 1
                oeng.dma_start(out=out[ob, :, lo:hi], in_=oot[:, lo:hi])
            if token is not None:
                # On sync, not Pool: Pool's SWDGE sem lanes are queue-locked
                # and must keep their periodic gather pattern when gq > 0.
                nc.sync.dma_start(out=token[:], in_=mw_sb[:1, 0, 0, :1])

    _split_sync_waits(nc)
    return nc


_NC_CACHE: bass.Bass | None = None


def _get_nc() -> bass.Bass:
    global _NC_CACHE
    if _NC_CACHE is None:
        _NC_CACHE = build_kernel()
    return _NC_CACHE


def make_in_maps(
    x: np.ndarray, style: np.ndarray, weight: np.ndarray, qn: int = 8, xlay: str = "c"
):
    qw = HW // qn
    # xlay="c": [B, CIN, HW] -> fp16 [B, qn, P, KT*qw]: chunk q / partition
    # p holds x[b, t*P + p, q*qw : (q+1)*qw] at offset t*qw — the layout
    # each chunk DMA consumes as one contiguous block.
    if xlay == "c":
        x_t = (
            np.asarray(x, dtype=np.float32)
            .reshape(B, KT, P, qn, qw)
            .transpose(0, 3, 2, 1, 4)
            .reshape(B, qn, P, KT * qw)
            .astype(np.float16)
        )
    else:
        x_t = np.asarray(x, dtype=np.float32).reshape(B, CIN, HW).astype(np.float16)
    # Identity gather indices: idx i is read from [i % 16, i // 16].
    gidx = np.zeros((P, P // 16), dtype=np.int16)
    for j in range(P // 16):
        gidx[:16, j] = np.arange(16, dtype=np.int16) + 16 * j
    styleT = np.ascontiguousarray(np.asarray(style, dtype=np.float32).T)  # [CIN, B]
    wT = np.ascontiguousarray(np.asarray(weight, dtype=np.float32).T)  # [CIN, COUT]
    in_maps = []
    for c in range(N_CORES):
        sl = slice(c * BPC, (c + 1) * BPC)
        in_maps.append(
            {
                "x": np.ascontiguousarray(x_t[sl]),
                "styleT": np.ascontiguousarray(styleT[:, sl]),
                "wT": wT,
                "gidx": gidx,
            }
        )
    return in_maps


def gather_out(results) -> np.ndarray:
    out = np.empty((B, COUT, H, W), dtype=np.float32)
    for c in range(N_CORES):
        out[c * BPC : (c + 1) * BPC] = (
            results[c]["out"].astype(np.float32).reshape(BPC, COUT, H, W)
        )
    return out


def kernel(x: np.ndarray, style: np.ndarray, weight: np.ndarray) -> np.ndarray:
    nc = _get_nc()
    in_maps = make_in_maps(x, style, weight)
    res = run_bass_kernel_spmd(nc, in_maps, core_ids=list(range(N_CORES)))
    return gather_out(res.results)
